# revision 29
# baseline (speedup 1.0000x reference)
"""CACombiner Trainium2 kernel: conv-projected efficient attention + FFN.

Data-parallel over batch: 8 batch elements -> 8 NeuronCores, identical SPMD
program per core.

Key tricks:
  - q/k/v projections as fp8e4m3 DoubleRow matmuls (K=256/instr).
  - Wr folded into the normalized context: WrCT = ctx_bd^T @ Wr^T computed
    once on-device, so reprojection is a single fp8 DoubleRow GEMM of the
    softmaxed q (stored fp8 x64) -- no att intermediate at all.
  - ELU via elu(x)+1 = max(x+1, min(e^x, 1)): the +1 rides the FFN1 bias
    fold, and the resulting he+1 offset is corrected by subtracting
    rowsum(W2) at the FFN2 eviction.
  - FFN in bf16 (fp8 fails the accuracy budget there); LN stats via
    ones-vector matmuls on bf16 copies.

Structure per core:
  phase 1  (16 x 256-l pairs): q softmax -> qsm8, exp(k), v, ctx/S accum
  phase 2a (8 x 512-l tiles):  fused reprojection + residual, LN1 -> zr
  phase 2b (8 x 512-l tiles):  FFN1 + ELU + FFN2, LN2 -> out
"""
import sys
sys.path.insert(0, "/opt/trn_rl_repo")
from contextlib import ExitStack

import numpy as np

import concourse.bass as bass
import concourse.tile as tile
from concourse import mybir, bacc
from concourse.bass_utils import run_bass_kernel_spmd
from concourse.alu_op_type import AluOpType

F32 = mybir.dt.float32
F32R = mybir.dt.float32r
BF16 = mybir.dt.bfloat16
F8 = mybir.dt.float8e4
AFT = mybir.ActivationFunctionType
Ax = mybir.AxisListType
DR = mybir.MatmulPerfMode.DoubleRow

B, C, L = 8, 512, 4096
H, DK = 8, 64
EPS = 1e-5
CC = C // 128           # 4 channel chunks
NP1 = L // 256          # 16 phase-1 pair-tiles (2x128 l)
NL2 = L // 512          # 8 phase-2 l-tiles

SW = 32.0               # weight scale for fp8
SA = 256.0              # ctx scale for bf16/fp8
SQ = 64.0               # qsm scale for fp8
ZDESC = 1.0 / (SA * SQ)  # descale for fused reprojection output
LN64 = float(np.log(64.0))

_CACHE = {}
LAST_RESULT = None


def _build_program(gates):
    (HAS_BQ, HAS_BK, HAS_BV, HAS_BR, HAS_B2, HAS_G2, HAS_BE2) = gates
    nc = bacc.Bacc("TRN2", target_bir_lowering=False, debug=False)

    def din(name, shape, dtype):
        return nc.dram_tensor(name, list(shape), dtype, kind="ExternalInput").ap()

    z1d = din("z1b", (C, L), BF16)
    z2d = din("z2", (C, L), F32)
    halfmask_d = din("halfmask", (128, CC, 8), BF16)
    ind64_d = din("ind64", (8, CC, 128), F32R)
    ident16k_d = din("ident16k", (128, 128), BF16)
    Wq8_d = din("Wq8", (128, CC, 512), F8)
    Wkv8_d = din("Wkv8", (128, CC, 1024), F8)
    WrTb_d = din("WrTb", (128, CC, 512), BF16)
    W1gb_d = din("W1gb", (128, CC, 1024), BF16)
    W2gb_d = din("W2gb", (128, 8, 512), BF16)
    U1f_d = din("U1f", (1, 2, 1024), F8)
    onesrow_d = din("onesrow", (1, 2, NL2, 512), F8)
    nw2s_c_d = din("nw2s_c", (128, CC), F32)
    inv512b_d = din("inv512b", (128, 1), BF16)
    ones1x128_d = din("ones1x128", (1, 128), F32R)
    identf8_d = din("identf8", (128, 128), F8)
    identb_d = din("identb", (128, 128), BF16)
    ones_f8_d = din("ones_f8", (128, 2, 1), F8)
    eps11_d = din("eps11", (1, 1), F32)
    negone_d = din("negone", (128, 1), F32)
    # gated bias constants (all-zero in the common case)
    bq32_d = din("bq32", (1, 512), F32R)
    bqsm_c_d = din("bqsm_c", (128, CC), F32)
    bk32_d = din("bk32", (1, 512), F32R)
    bv_c_d = din("bv_c", (128, CC), F32)
    br_c_d = din("br_c", (128, CC), F32)
    g2_c_d = din("g2_c", (128, CC), F32)
    be2_c_d = din("be2_c", (128, CC), F32)
    outd = nc.dram_tensor("out", [C, L], F32, kind="ExternalOutput").ap()

    z1r = z1d.rearrange("(cc p) l -> p cc l", p=128)
    z2r = z2d.rearrange("(cc p) l -> p cc l", p=128)
    outr = outd.rearrange("(cc p) l -> p cc l", p=128)

    mm = nc.tensor.matmul
    tt = nc.vector.tensor_tensor
    ts = nc.vector.tensor_scalar
    stt = nc.vector.scalar_tensor_tensor
    act = nc.scalar.activation
    pts = nc.gpsimd.tensor_scalar
    pstt = nc.gpsimd.scalar_tensor_tensor
    ptt = nc.gpsimd.tensor_tensor
    pcopy = nc.gpsimd.tensor_copy

    with tile.TileContext(nc) as tc, ExitStack() as ctx:
        cpool = ctx.enter_context(tc.tile_pool(name="consts", bufs=1))

        def const_tile(shape, dtype, src, tag, defer=False):
            t = cpool.tile(list(shape), dtype, tag=tag, name=tag)
            if defer:
                deferred_dmas.append((t, src))
            else:
                nc.scalar.dma_start(t[:], src)
            return t

        deferred_dmas = []
        # loaded up front: everything phase 1 touches
        ones_f8 = const_tile((128, 2, 1), F8, ones_f8_d, "ones_f8")
        Wkv8 = const_tile((128, CC, 1024), F8, Wkv8_d, "Wkv8")
        # loaded during phase 1 (consumed by phase 2)
        Wq8 = const_tile((128, CC, 512), F8, Wq8_d, "Wq8", defer=True)
        halfmask = const_tile((128, CC, 8), BF16, halfmask_d, "halfmask",
                              defer=True)
        ind64 = const_tile((8, CC, 128), F32R, ind64_d, "ind64", defer=True)
        ident16k = const_tile((128, 128), BF16, ident16k_d, "ident16k",
                              defer=True)
        identb = const_tile((128, 128), BF16, identb_d, "identb", defer=True)
        inv512b = const_tile((128, 1), BF16, inv512b_d, "inv512b", defer=True)
        ones1x128 = const_tile((1, 128), F32R, ones1x128_d, "ones1x128",
                               defer=True)
        eps11 = const_tile((1, 1), F32, eps11_d, "eps11", defer=True)
        negone = const_tile((128, 1), F32, negone_d, "negone", defer=True)
        WrTb = const_tile((128, CC, 512), BF16, WrTb_d, "WrTb", defer=True)
        W1gb = const_tile((128, CC, 1024), BF16, W1gb_d, "W1gb", defer=True)
        W2gb = const_tile((128, 8, 512), BF16, W2gb_d, "W2gb", defer=True)
        U1f = const_tile((1, 2, 1024), F8, U1f_d, "U1f", defer=True)
        nw2s_c = const_tile((128, CC), F32, nw2s_c_d, "nw2s_c", defer=True)
        if HAS_BQ:
            bq32 = const_tile((1, 512), F32R, bq32_d, "bq32")
            bqsm_c = const_tile((128, CC), F32, bqsm_c_d, "bqsm_c",
                                defer=True)
        if HAS_BK:
            bk32 = const_tile((1, 512), F32R, bk32_d, "bk32")
        if HAS_BV:
            bv_c = const_tile((128, CC), F32, bv_c_d, "bv_c")
        if HAS_BR:
            br_c = const_tile((128, CC), F32, br_c_d, "br_c")
        if HAS_G2:
            g2_c = const_tile((128, CC), F32, g2_c_d, "g2_c")
        if HAS_BE2:
            be2_c = const_tile((128, CC), F32, be2_c_d, "be2_c")

        # persistent across phases
        WrCT8 = cpool.tile([128, CC, 512], F8, tag="WrCT8", name="WrCT8")
        zr_all = cpool.tile([128, NL2, CC, 512], BF16, tag="zr", name="zr_all")
        mu_all = cpool.tile([1, NL2, 512], BF16, tag="mu1", name="mu_all")
        e2_all = cpool.tile([1, NL2, 512], BF16, tag="e21", name="e2_all")
        mur_all = cpool.tile([1, 2, NL2, 512], F8, tag="mur", name="mur_all")
        deferred_dmas.append((mur_all[:], onesrow_d))

        # ---------------- Phase 1: exp(k), v, ctx/S accumulation -------------
        with ExitStack() as p1:
            lp = p1.enter_context(tc.tile_pool(name="lp1", bufs=3))
            psw = p1.enter_context(tc.tile_pool(name="psw", bufs=3, space="PSUM"))
            psc = p1.enter_context(tc.tile_pool(name="psc", bufs=1, space="PSUM"))

            ctxps = psc.tile([128, CC, 128], F32, tag="ctxps", name="ctxps")
            Sps = psc.tile([128, CC], F32, tag="Sps", name="Sps")

            for p in range(NP1):
                l0 = p * 256
                sl = slice(l0, l0 + 256)
                z2c = lp.tile([128, CC, 256], F32, tag="z2c")
                nc.sync.dma_start(z2c[:], z2r[:, :, sl])
                z2f8 = lp.tile([128, CC, 256], F8, tag="z2f8")
                pcopy(z2f8[:], z2c[:])

                # k fp8 DoubleRow (values = SW * k_true)
                kps = psw.tile([128, 2, 512], F32, tag="pw", name="kps")
                for i in range(2):
                    ls = slice(i * 128, (i + 1) * 128)
                    mm(kps[:, i, :], z2f8[:, 0:2, ls], Wkv8[:, 0:2, 0:512],
                       start=True, stop=False, perf_mode=DR)
                    mm(kps[:, i, :], z2f8[:, 2:4, ls], Wkv8[:, 2:4, 0:512],
                       start=False, stop=not HAS_BK, perf_mode=DR)
                    if HAS_BK:
                        mm(kps[:, i, :], ones1x128[:], bk32[:],
                           start=False, stop=True)
                EkT = lp.tile([128, 2, 512], F8, tag="EkT")
                act(EkT[:], kps[:], AFT.Exp, scale=1.0 / SW)

                # v fp8 DoubleRow
                vps = psw.tile([128, 2, 512], F32, tag="pw", name="vps")
                for i in range(2):
                    ls = slice(i * 128, (i + 1) * 128)
                    mm(vps[:, i, :], z2f8[:, 0:2, ls], Wkv8[:, 0:2, 512:1024],
                       start=True, stop=False, perf_mode=DR)
                    mm(vps[:, i, :], z2f8[:, 2:4, ls], Wkv8[:, 2:4, 512:1024],
                       start=False, stop=True, perf_mode=DR)
                vT = lp.tile([128, 2, 512], F8, tag="vT")
                if HAS_BV:
                    for cc in range(CC):
                        cs = slice(cc * 128, (cc + 1) * 128)
                        ts(vT[:, :, cs], vps[:, :, cs], 1.0 / SW,
                           bv_c[:, cc:cc + 1], AluOpType.mult, AluOpType.add)
                else:
                    ts(vT[:], vps[:], 1.0 / SW, None, AluOpType.mult)

                # ctx/S accumulation over l
                for pr in range(CC):
                    ks = slice(pr * 128, (pr + 1) * 128)
                    mm(ctxps[:, pr, :], EkT[:, :, ks], vT[:, :, ks],
                       start=(p == 0), stop=(p == NP1 - 1), perf_mode=DR,
                       skip_group_check=True)
                    mm(Sps[:, pr:pr + 1], EkT[:, :, ks], ones_f8[:],
                       start=(p == 0), stop=(p == NP1 - 1), perf_mode=DR,
                       skip_group_check=True)

                if p == 0 and deferred_dmas:
                    for _t, _src in deferred_dmas:
                        _ap = _t[:] if hasattr(_t, "tile") else _t
                        nc.scalar.dma_start(_ap, _src)
                    deferred_dmas = []

            # finalize: ctx_bd = (ctx / S) * SA block-diagonal bf16, then
            # fold Wr: WrCT8[k, o] = sum_v ctx_bd[k, v] * WrT[v, o]  (fp8)
            rs = lp.tile([128, CC], F32, tag="rs", bufs=1)
            nc.vector.reciprocal(rs[:], Sps[:])
            ctxbd = lp.tile([128, CC, 128], BF16, tag="ctxbd", bufs=1)
            ctxbdT = lp.tile([128, CC, 128], BF16, tag="ctxbdT", bufs=1)
            nc.vector.memset(ctxbd[:], 0.0)
            for pr in range(CC):
                for h2 in range(2):
                    s = slice(h2 * 64, (h2 + 1) * 64)
                    ts(ctxbd[s, pr, s], ctxps[s, pr, s], rs[s, pr:pr + 1], SA,
                       AluOpType.mult, AluOpType.mult)
            # (bv, if present, was already folded into v at the vT eviction)
            tpsT = psw.tile([128, CC, 128], BF16, tag="pw", name="tpsT")
            for pr in range(CC):
                nc.tensor.transpose(tpsT[:, pr, :], ctxbd[:, pr, :], identb[:])
            nc.vector.tensor_copy(ctxbdT[:], tpsT[:])
            for half in range(2):
                wps = psw.tile([128, 2, 512], F32, tag="pw", name="wps")
                for i in range(2):
                    pr = half * 2 + i
                    mm(wps[:, i, :], ctxbdT[:, pr, :], WrTb[:, pr, :],
                       start=True, stop=True)
                ts(WrCT8[:, half * 2:half * 2 + 2, :], wps[:], 1.0, None,
                   AluOpType.mult)

        # ------------- Phase 2a: q softmax + fused reprojection + LN1 --------
        with ExitStack() as p2a:
            lpa = p2a.enter_context(tc.tile_pool(name="lpa", bufs=3))
            psb = p2a.enter_context(tc.tile_pool(name="psb", bufs=2, space="PSUM"))
            pst = p2a.enter_context(tc.tile_pool(name="pst", bufs=2, space="PSUM"))
            psr = p2a.enter_context(tc.tile_pool(name="psr", bufs=2, space="PSUM"))

            st = [dict() for _ in range(NL2)]

            def a_f1(t):
                """z1 load + fp8 + channel-major q projection + exp + S"""
                s = st[t]
                sl = slice(t * 512, (t + 1) * 512)
                z1res = lpa.tile([128, CC, 512], BF16, tag="z1res", bufs=3,
                                 name="z1res")
                nc.sync.dma_start(z1res[:], z1r[:, :, sl])
                z1f8 = lpa.tile([128, CC, 512], F8, tag="z1f8", name="z1f8")
                pcopy(z1f8[:, 0:2, :], z1res[:, 0:2, :])
                pcopy(z1f8[:, 2:4, :], z1res[:, 2:4, :])
                s["z1res"] = z1res
                Eqcf = lpa.tile([128, CC, 512], BF16, tag="Eqcf", bufs=2,
                                name="Eqcf")
                for g in range(2):
                    qcm = psb.tile([128, 2, 512], F32, tag="qps", bufs=1,
                                   name="qcm")
                    for i in range(2):
                        oc = g * 2 + i
                        os_ = slice(oc * 128, (oc + 1) * 128)
                        mm(qcm[:, i, :], Wq8[:, 0:2, os_], z1f8[:, 0:2, :],
                           start=True, stop=False, perf_mode=DR)
                        mm(qcm[:, i, :], Wq8[:, 2:4, os_], z1f8[:, 2:4, :],
                           start=False, stop=True, perf_mode=DR)
                    if HAS_BQ:
                        for i in range(2):
                            oc = g * 2 + i
                            act(Eqcf[:, oc, :], qcm[:, i, :], AFT.Exp,
                                scale=1.0 / SW, bias=bqsm_c[:, oc:oc + 1])
                    else:
                        act(Eqcf[:, g * 2:g * 2 + 2, :], qcm[:], AFT.Exp,
                            scale=1.0 / SW)
                Sp = psr.tile([8, 512], F32, tag="row", name="Sp")
                for cc in range(CC):
                    mm(Sp[:], halfmask[:, cc, :], Eqcf[:, cc, :],
                       start=(cc == 0), stop=(cc == CC - 1))
                s["Eqcf"], s["Sp"] = Eqcf, Sp

            def a_f2(t):
                """rq = 64/S broadcast via PE; qsm8 = Eq * rq (fp8)"""
                s = st[t]
                rqc = lpa.tile([8, 512], F32R, tag="rqc", name="rqc")
                with nc.allow_low_precision(reason="f32r rows for PE"):
                    nc.vector.reciprocal(rqc[:], s["Sp"][:])
                qsm8t = lpa.tile([128, CC, 512], F8, tag="qsm8t", bufs=3,
                                 name="qsm8t")
                for cc in range(CC):
                    rqb = psb.tile([128, 512], F32, tag="rqb", bufs=2,
                                   name="rqb")
                    mm(rqb[:], ind64[:, cc, :], rqc[:],
                       start=True, stop=True)
                    tt(qsm8t[:, cc, :], s["Eqcf"][:, cc, :], rqb[:],
                       AluOpType.mult)
                s["qsm8t"] = qsm8t

            def a_mid(t):
                """fused reprojection + residual (via PE) + LN1 stats"""
                s = st[t]
                qsm8t = s["qsm8t"]
                z = zr_all[:, t, :, :]
                for half in range(2):
                    zps = psb.tile([128, 2, 512], F32, tag="zps", bufs=1,
                                   name="zps")
                    for i in range(2):
                        ot = half * 2 + i
                        os_ = slice(ot * 128, (ot + 1) * 128)
                        mm(zps[:, i, :], WrCT8[:, 0:2, os_], qsm8t[:, 0:2, :],
                           start=True, stop=False, perf_mode=DR)
                        mm(zps[:, i, :], WrCT8[:, 2:4, os_], qsm8t[:, 2:4, :],
                           start=False, stop=False, perf_mode=DR)
                        mm(zps[:, i, :], ident16k[:], s["z1res"][:, ot, :],
                           start=False, stop=True)
                    hs = slice(half * 2, half * 2 + 2)
                    if half == 0:
                        ts(z[:, hs, :], zps[:], ZDESC, None, AluOpType.mult)
                    else:
                        act(z[:, hs, :], zps[:], AFT.Copy, scale=ZDESC)
                    if HAS_BR:
                        for i in range(2):
                            cc = half * 2 + i
                            ts(z[:, cc, :], z[:, cc, :], br_c[:, cc:cc + 1],
                               None, AluOpType.add)
                zsq = lpa.tile([128, CC, 512], BF16, tag="zsq", name="zsq")
                tt(zsq[:], z[:], z[:], AluOpType.mult)
                mups = psr.tile([1, 512], F32, tag="row", name="mups")
                for cc in range(CC):
                    mm(mups[:], inv512b[:], z[:, cc, :], start=(cc == 0),
                       stop=(cc == CC - 1))
                e2ps = psr.tile([1, 512], F32, tag="row", name="e2ps")
                for cc in range(CC):
                    mm(e2ps[:], inv512b[:], zsq[:, cc, :], start=(cc == 0),
                       stop=(cc == CC - 1))
                act(mu_all[0:1, t, :], mups[:], AFT.Copy)
                act(e2_all[0:1, t, :], e2ps[:], AFT.Copy)
                st[t] = {}

            for t in range(NL2 + 2):
                if t < NL2:
                    a_f1(t)
                if 1 <= t <= NL2:
                    a_f2(t - 1)
                if 2 <= t <= NL2 + 1:
                    a_mid(t - 2)

        # ------------- Phase 2b: FFN1 + ELU + FFN2 + LN2 -> out --------------
        with ExitStack() as p2b:
            lpb = p2b.enter_context(tc.tile_pool(name="lpb", bufs=2))
            lph = p2b.enter_context(tc.tile_pool(name="lph", bufs=1))
            psF = p2b.enter_context(tc.tile_pool(name="psF", bufs=1, space="PSUM"))
            psf = p2b.enter_context(tc.tile_pool(name="psf", bufs=2, space="PSUM"))
            psr2 = p2b.enter_context(tc.tile_pool(name="psr2", bufs=2, space="PSUM"))

            sb = [dict() for _ in range(NL2)]
            bk = [dict() for _ in range(NL2)]

            def b_back(u, step):
                """LN1 rows for tile u (rsqrt + r-broadcast + in-place zr)"""
                s = bk[u]
                if step == 0:
                    musq = lpb.tile([1, 512], F32, tag="musq1", bufs=2,
                                    name="musq")
                    ptt(musq[:], mu_all[0:1, u, :], mu_all[0:1, u, :],
                        AluOpType.mult)
                    varrow = lpb.tile([1, 512], F32, tag="varrow", bufs=2,
                                      name="varrow")
                    ptt(varrow[:], e2_all[0:1, u, :], musq[:],
                        AluOpType.subtract)
                    s["varrow"] = varrow
                elif step == 1:
                    sig = lpb.tile([1, 512], F32, tag="sig", bufs=2,
                                   name="sig")
                    act(sig[:], s["varrow"][:], AFT.Sqrt, bias=eps11[0:1, :])
                    s["sig"] = sig
                elif step == 2:
                    rrow = lpb.tile([1, 512], F32R, tag="rrow", bufs=2,
                                    name="rrow")
                    with nc.allow_low_precision(reason="f32r rows for PE"):
                        nc.vector.reciprocal(rrow[:], s["sig"][:])
                    pstt(mur_all[0:1, 0, u, :], mu_all[0:1, u, :], 8.0,
                         rrow[:], AluOpType.mult, AluOpType.mult)
                    s["rrow"] = rrow
                elif step == 3:
                    rbc = psr2.tile([128, 512], F32, tag="row", name="rbc")
                    mm(rbc[:], ones1x128[:], s["rrow"][:],
                       start=True, stop=True)
                    rbcb = lpb.tile([128, 512], BF16, tag="rbcb", name="rbcb")
                    act(rbcb[:], rbc[:], AFT.Copy)
                    tt(zr_all[:, u, :, :], zr_all[:, u, :, :],
                       rbcb[:].unsqueeze(1).broadcast_to([128, CC, 512]),
                       AluOpType.mult)
                    bk[u] = {}

            def b_post(t, step):
                """LN2 pieces of tile t, emitted interleaved with tile t+1."""
                s = sb[t]
                if step == 0:
                    y = lpb.tile([128, CC, 512], BF16, tag="y", name="y")
                    for cc in range(CC):
                        if cc % 2 == 0:
                            ts(y[:, cc, :], s["f2ps"][:, cc, :],
                               nw2s_c[:, cc:cc + 1], None, AluOpType.add)
                        else:
                            act(y[:, cc, :], s["f2ps"][:, cc, :], AFT.Identity,
                                bias=nw2s_c[:, cc:cc + 1])
                    s["y"] = y
                elif step == 1:
                    ysq = lpb.tile([128, CC, 512], BF16, tag="ysq", name="ysq")
                    tt(ysq[:], s["y"][:], s["y"][:], AluOpType.mult)
                    muy = psr2.tile([1, 512], F32, tag="row", name="muy")
                    for cc in range(CC):
                        mm(muy[:], inv512b[:], s["y"][:, cc, :],
                           start=(cc == 0), stop=(cc == CC - 1))
                    s["ysq"], s["muy"] = ysq, muy
                elif step == 2:
                    e2y = psr2.tile([1, 512], F32, tag="row", name="e2y")
                    for cc in range(CC):
                        mm(e2y[:], inv512b[:], s["ysq"][:, cc, :],
                           start=(cc == 0), stop=(cc == CC - 1))
                    s["e2y"] = e2y
                elif step == 3:
                    mur2 = lpb.tile([1, 512], F32R, tag="mur2", bufs=2,
                                    name="mur2")
                    nc.vector.tensor_copy(mur2[:], s["muy"][:])
                    musq2 = lpb.tile([1, 512], F32, tag="musq2", bufs=2,
                                     name="musq2")
                    ptt(musq2[:], mur2[:], mur2[:], AluOpType.mult)
                    s["mur2"], s["musq2"] = mur2, musq2
                elif step == 4:
                    var2 = lpb.tile([1, 512], F32, tag="var2", bufs=2,
                                    name="var2")
                    tt(var2[:], s["e2y"][:], s["musq2"][:], AluOpType.subtract)
                    sig2 = lpb.tile([1, 512], F32, tag="sig2", bufs=2,
                                    name="sig2")
                    act(sig2[:], var2[:], AFT.Sqrt, bias=eps11[0:1, :])
                    s["sig2"] = sig2
                elif step == 5:
                    r2row = lpb.tile([1, 512], F32R, tag="r2row", bufs=2,
                                     name="r2row")
                    with nc.allow_low_precision(reason="f32r rows for PE"):
                        nc.vector.reciprocal(r2row[:], s["sig2"][:])
                    s["r2row"] = r2row
                elif step == 6:
                    r2bc = psr2.tile([128, 512], F32, tag="row", name="r2bc")
                    mm(r2bc[:], ones1x128[:], s["r2row"][:],
                       start=True, stop=True)
                    mu2bc = psr2.tile([128, 512], F32, tag="row", name="mu2bc")
                    mm(mu2bc[:], ones1x128[:], s["mur2"][:],
                       start=True, stop=True)
                    mu2b = lpb.tile([128, 512], BF16, tag="mu2b", name="mu2b")
                    act(mu2b[:], mu2bc[:], AFT.Copy)
                    s["r2bc"], s["mu2b"] = r2bc, mu2b
                elif step == 7:
                    sl = slice(t * 512, (t + 1) * 512)
                    yc = lpb.tile([128, CC, 512], BF16, tag="yc", name="yc")
                    tt(yc[:], s["y"][:],
                       s["mu2b"][:].unsqueeze(1).broadcast_to([128, CC, 512]),
                       AluOpType.subtract)
                    outt = lpb.tile([128, CC, 512], F32, tag="outt",
                                    name="outt")
                    tt(outt[:], yc[:],
                       s["r2bc"][:].unsqueeze(1).broadcast_to([128, CC, 512]),
                       AluOpType.mult)
                    if HAS_G2:
                        for cc in range(CC):
                            ts(outt[:, cc, :], outt[:, cc, :],
                               g2_c[:, cc:cc + 1], None, AluOpType.mult)
                    if HAS_BE2:
                        for cc in range(CC):
                            ts(outt[:, cc, :], outt[:, cc, :],
                               be2_c[:, cc:cc + 1], None, AluOpType.add)
                    nc.sync.dma_start(outr[:, :, sl], outt[:])
                    sb[t] = {}

            for step in range(4):
                b_back(0, step)
            for t in range(NL2):
                f2ps = psF.tile([128, CC, 512], F32, tag="f2", name="f2ps")
                sb[t]["f2ps"] = f2ps
                hes = [None] * 8
                for j in range(8):
                    js = slice(j * 128, (j + 1) * 128)
                    fps = psf.tile([128, 512], F32, tag="fps", name="fps")
                    for cc in range(CC):
                        mm(fps[:], W1gb[:, cc, js], zr_all[:, t, cc, :],
                           start=(cc == 0), stop=False)
                    # fps = h + 1 (row0: -u1/8 x 8*mu*r ; row1: (w1bb+1) x 1)
                    mm(fps[:], U1f[:, :, js], mur_all[:, :, t, :],
                       start=False, stop=True, perf_mode=DR)
                    E = lpb.tile([128, 512], BF16, tag="E", name="E")
                    act(E[:], fps[:], AFT.Exp, bias=negone[:, 0:1])
                    he = lph.tile([128, 512], BF16, tag="he", bufs=4, name="he")
                    # he = elu(h)+1 = max(h+1, min(e^h, 1))
                    stt(he[:], E[:], 1.0, fps[:], AluOpType.min,
                        AluOpType.max)
                    hes[j] = he
                    if j > 1:
                        jp = j - 2
                        for o2 in range(CC):
                            mm(f2ps[:, o2, :],
                               W2gb[:, jp, o2 * 128:(o2 + 1) * 128],
                               hes[jp][:], start=(jp == 0), stop=False,
                               skip_group_check=True)
                    if t > 0:
                        b_post(t - 1, j)
                    if t + 1 < NL2 and j in (2, 4, 5, 7):
                        b_back(t + 1, {2: 0, 4: 1, 5: 2, 7: 3}[j])
                for jp in (6, 7):
                    for o2 in range(CC):
                        mm(f2ps[:, o2, :], W2gb[:, jp, o2 * 128:(o2 + 1) * 128],
                           hes[jp][:], start=False, stop=(jp == 7),
                           skip_group_check=True)
            for step in range(8):
                b_post(NL2 - 1, step)

    nc.compile()
    return nc


def _mk_halfmask(bf):
    m = np.zeros((128, CC, 8), np.float32)
    for cc in range(CC):
        for p in range(128):
            m[p, cc, 2 * cc + (p >= 64)] = 1.0
    return m.astype(bf)


def _mk_ind64(f):
    m = np.zeros((8, CC, 128), np.float32)
    for cc in range(CC):
        m[2 * cc, cc, 0:64] = SQ
        m[2 * cc + 1, cc, 64:128] = SQ
    return m.astype(f)


def _prep_consts(Wq, bq, Wk, bk, Wv, bv, Wr, br, g1, be1, W1, b1, W2, b2, g2, be2):
    import ml_dtypes
    f = np.float32
    f8 = ml_dtypes.float8_e4m3
    bf = ml_dtypes.bfloat16

    def chunkT(a, n):          # [n*128, m] -> [128, n, m]
        return np.ascontiguousarray(a.reshape(n, 128, -1).transpose(1, 0, 2))

    def colsT(v, n):           # [n*128] -> [128, n]
        return np.ascontiguousarray(v.reshape(n, 128).T)

    WqT = np.ascontiguousarray(Wq.T, dtype=f)                       # [c, o]
    WkvT = np.concatenate([Wk.T, Wv.T], axis=1).astype(f)           # [c, k|v]
    WrT = np.ascontiguousarray(Wr.T, dtype=f)                       # [v, o]
    g2_is_one = bool(np.all(g2 == 1.0))
    W1g = (W1 * g1[None, :]).astype(f)                              # [1024, c]
    W2u = (W2 * g2[:, None]).astype(f) if g2_is_one else W2.astype(f)
    u1 = W1g.sum(axis=1).astype(f)
    w1bb = (W1 @ be1 + b1).astype(f)
    w2s = W2u.sum(axis=1).astype(f)                                 # rowsum(W2)

    gates = (
        bool(np.any(bq != 0)), bool(np.any(bk != 0)), bool(np.any(bv != 0)),
        bool(np.any(br != 0)),
        not g2_is_one, bool(np.any(b2 != 0)), bool(np.any(be2 != 0)),
    )
    # gates order used by build: BQ, BK, BV, BR, B2?? -> include b2 into nw2s
    gates = (gates[0], gates[1], gates[2], gates[3], gates[5], gates[4],
             gates[6])
    # nw2s absorbs -rowsum(W2) and +b2
    nw2s = (b2 - w2s).astype(f)

    consts = {
        "Wq8": chunkT(WqT * SW, CC).astype(f8),
        "Wkv8": chunkT(WkvT * SW, CC).astype(f8),
        "WrTb": chunkT(WrT, CC).astype(bf),
        "W1gb": chunkT(np.ascontiguousarray(W1g.T), CC).astype(bf),
        "W2gb": chunkT(np.ascontiguousarray(W2u.T), 8).astype(bf),
        "U1f": np.stack([-u1 / 8.0, w1bb + 1.0])[None].astype(f8),
        "onesrow": np.stack([np.zeros((NL2, 512), np.float32),
                             np.ones((NL2, 512), np.float32)])[None].astype(f8),
        "nw2s_c": colsT(nw2s, CC),
        "halfmask": _mk_halfmask(bf),
        "ind64": _mk_ind64(f),
        "ident16k": (np.eye(128) * (SA * SQ)).astype(bf),
        "inv512b": np.full((128, 1), 1.0 / 512.0, dtype=bf),
        "ones1x128": np.ones((1, 128), dtype=f),
        "identf8": np.eye(128, dtype=f8),
        "identb": np.eye(128, dtype=bf),
        "ones_f8": np.ones((128, 2, 1), dtype=f8),
        "eps11": np.full((1, 1), EPS, dtype=f),
        "negone": np.full((128, 1), -1.0, dtype=f),
        "bq32": (bq * SW).reshape(1, 512).astype(f),
        "bqsm_c": colsT((bq * SW).astype(f), CC),
        "bk32": (bk * SW).reshape(1, 512).astype(f),
        "bv_c": colsT(bv.astype(f), CC),
        "br_c": colsT(br.astype(f), CC),
        "g2_c": colsT(g2.astype(f), CC),
        "be2_c": colsT(be2.astype(f), CC),
    }
    return consts, gates


def kernel(**inputs):
    global LAST_RESULT
    import ml_dtypes
    z1 = np.asarray(inputs["z1"], dtype=np.float32)
    z2 = np.asarray(inputs["z2"], dtype=np.float32)
    consts, gates = _prep_consts(
        *[np.asarray(inputs[k], dtype=np.float32) for k in
          ["Wq", "bq", "Wk", "bk", "Wv", "bv", "Wr", "br", "g1", "be1",
           "W1", "b1", "W2", "b2", "g2", "be2"]])

    key = ("prog", gates)
    if key not in _CACHE:
        _CACHE.clear()
        _CACHE[key] = _build_program(gates)
    nc = _CACHE[key]

    in_maps = []
    for b in range(B):
        m = dict(consts)
        m["z1b"] = np.ascontiguousarray(z1[b]).astype(ml_dtypes.bfloat16)
        m["z2"] = np.ascontiguousarray(z2[b])
        in_maps.append(m)

    import os
    trace = bool(int(os.environ.get("KERNEL_TRACE", "0")))
    res = run_bass_kernel_spmd(nc, in_maps, list(range(B)), trace=trace)
    LAST_RESULT = res
    out = np.stack([res.results[b]["out"] for b in range(B)], axis=0)
    return out.astype(np.float32)
